# revision 16
# baseline (speedup 1.0000x reference)
"""DecoderLSTM Trainium2 kernel — 8-core data-parallel over batch.

Problem: 2-layer LSTM (H=512, B=512, T=128) where the step input is the sum of
the two layers' hidden states, followed by a 3-layer MLP head applied to the
[B, T, H] hidden-sum sequence.

Strategy (per core, B_c = 64 batch rows, zero collectives):
  - LSTM gates computed as g[B_c, 4H] with the *activations* stationary on the
    PE array ([K=128, M=64] tiles of x^T / h^T) and the *weights* streaming as
    the moving operand in fp32r (full-rate, ~1.5e-4 numerics) 512-col chunks.
  - h_new is transposed back to [H, B_c] each step with PE transpose-mode
    matmuls so the next step's stationary operands need no extra work.
  - Hidden sums are staged transposed in SBUF rings and flushed to a DRAM
    scratch every 8 steps; the MLP head then runs chunk-wise (512 rows at a
    time): fc1/fc2 weights-stationary with fused bias+ReLU on the scalar
    engine, fc3 activations-stationary so the result lands in [rows, H] layout
    for direct output DMA.
  - Raw bass (no Tile): explicit per-engine programs and semaphores.
"""

import ml_dtypes
import numpy as np

import concourse.bass as bass
import concourse.mybir as mybir
from concourse.bass_utils import run_bass_kernel_spmd

F32 = mybir.dt.float32
F32R = mybir.dt.float32r
BF16 = mybir.dt.bfloat16
AF = mybir.ActivationFunctionType
MUL = mybir.AluOpType.mult
ADD = mybir.AluOpType.add

NCORES = 8
B, H, T, L = 512, 512, 128, 2
BC = B // NCORES          # 64 batch rows per core
G = 4 * H                 # 2048 gate rows
KT = H // 128             # 4 K-tiles
NCH = 4                   # gate column chunks of 512
RING = 8                  # steps per outs ring flush
NFLUSH = T // RING        # 16
NCHUNK = (BC * T) // 512  # 16 MLP row chunks of 512

# ---- semaphore value schedules (pure functions of step/chunk) ----------------
# dve events per LSTM step: c0, h0, hT0, c1, h1, hT1, hsum  (1-based)
_DVE_E = {"c0": 1, "h0": 2, "hT0": 3, "c1": 4, "h1": 5, "hT1": 6, "hsum": 7}
# act events per LSTM step: sig0, tang0, tanc0, sig1, tang1, tanc1
_ACT_E = {"sig0": 1, "tang0": 2, "tanc0": 3, "sig1": 4, "tang1": 5, "tanc1": 6}
# pe events per LSTM step: l0c012, l0c3, l0T, l1c012, l1c3, l1T
_PE_E = {"l0c012": 1, "l0c3": 2, "l0T": 3, "l1c012": 4, "l1c3": 5, "l1T": 6}


def dve_v(t, e):
    return 1 + 7 * t + _DVE_E[e]          # +1 for initial xT copy


def act_v(t, e):
    return 6 * t + _ACT_E[e]


def pe_v(t, e):
    return 1 + 6 * t + _PE_E[e]           # +1 for x transposes


DVE_LSTM_END = 1 + 7 * T
ACT_LSTM_END = 6 * T
PE_LSTM_END = 1 + 6 * T


def dve_mlp(j, m):                        # after o3 copy (j, rowm m)
    return DVE_LSTM_END + 4 * j + m + 1


def act_mlp(j, which):                    # which: 1 = relu1, 2 = relu2
    return ACT_LSTM_END + 2 * j + which


# pe events per MLP chunk: fc1, fc2, fc3m0..fc3m3
def pe_mlp(j, e):
    base = PE_LSTM_END + 6 * j
    if e == "fc1":
        return base + 1
    if e == "fc2":
        return base + 2
    return base + 3 + int(e)


NPRE = 16                                 # preload DMA count
DMA_IN_PRE = 16 * NPRE


def dma_in_load(j):                       # after MLP chunk-j actT load
    return DMA_IN_PRE + 16 * (j + 1)


def dma_out_flush(f):
    return 16 * (f + 1)


def dma_out_mlp(j, m):
    return 16 * (NFLUSH + 4 * j + m + 1)


def build_nc():
    nc = bass.Bass("TRN2", target_bir_lowering=False, debug=False,
                   num_devices=NCORES)

    # ---- DRAM I/O ----
    x_d = nc.dram_tensor("x", [BC, H], F32, kind="ExternalInput")
    wih_d = [nc.dram_tensor(f"wih{l}", [KT, 128, G], F32, kind="ExternalInput") for l in range(L)]
    whh_d = [nc.dram_tensor(f"whh{l}", [KT, 128, G], F32, kind="ExternalInput") for l in range(L)]
    bias_d = [nc.dram_tensor(f"bias{l}", [1, G], BF16, kind="ExternalInput") for l in range(L)]
    fc1w_d = nc.dram_tensor("fc1w", [KT, 128, 512], F32, kind="ExternalInput")
    fc2w_d = nc.dram_tensor("fc2w", [KT, 128, 512], F32, kind="ExternalInput")
    fc3w_d = nc.dram_tensor("fc3w", [KT, 128, 512], F32, kind="ExternalInput")
    fc1b_d = nc.dram_tensor("fc1b", [128, 4], F32, kind="ExternalInput")
    fc2b_d = nc.dram_tensor("fc2b", [128, 4], F32, kind="ExternalInput")
    fc3b_d = nc.dram_tensor("fc3b", [1, 512], F32, kind="ExternalInput")
    ones64_d = nc.dram_tensor("ones64", [1, 64], BF16, kind="ExternalInput")
    ones128_d = nc.dram_tensor("ones128", [1, 128], F32, kind="ExternalInput")
    id64_d = nc.dram_tensor("id64", [64, 64], F32, kind="ExternalInput")
    outsT_d = nc.dram_tensor("outsT", [KT, 128, T, BC], F32, kind="Internal")
    out_d = nc.dram_tensor("out", [BC, T, H], F32, kind="ExternalOutput")

    # ---- hand-drawn SBUF map (per-partition byte offsets) ----
    off = [(nc.sbuf_base + 63) // 64 * 64]

    def at(name, shape, dtype, align=32, offset=None):
        o = (off[0] + align - 1) // align * align if offset is None else offset
        h = nc.alloc_sbuf_tensor_at(name, shape, dtype, offset=o)
        sz = int(np.prod(shape[1:])) * mybir.dt.size(dtype)
        if offset is None:
            off[0] = o + sz
        return h

    wih = [at(f"wih{l}s", [128, KT, G], F32R) for l in range(L)]
    whh = [at(f"whh{l}s", [128, KT, G], F32R) for l in range(L)]
    fc1w = at("fc1ws", [128, KT, 512], F32R)
    fc2w = at("fc2ws", [128, KT, 512], F32R)
    fc3w = at("fc3ws", [128, KT, 512], F32R)
    biasr = [at(f"bias{l}s", [1, G], BF16) for l in range(L)]
    fc3br = at("fc3bs", [1, 512], F32R)
    ones64 = at("ones64s", [1, 64], BF16)
    ones128 = at("ones128s", [1, 128], F32R)
    id64 = at("id64s", [64, 64], F32)
    fc1b = at("fc1bs", [128, 4], F32)
    fc2b = at("fc2bs", [128, 4], F32)

    lstm_base = off[0]
    ring = [at(f"ring{r}", [128, KT, RING, BC], F32) for r in range(2)]
    sig = at("sig", [64, 1536], F32)
    tang = at("tang", [64, 512], F32)
    tanc = at("tanc", [64, 512], F32)
    hnew = at("hnew", [64, 512], F32)
    tmp = at("tmp", [64, 512], F32)
    c_sb = [at(f"c{l}", [64, 512], F32) for l in range(L)]
    hsumT = at("hsumT", [128, KT, BC], F32R)
    h0T = at("h0T", [128, KT, BC], F32R)
    h1T = at("h1T", [128, KT, BC], F32R)
    # x_sb and xT are dead after step 0 starts: alias onto ring0 / hsumT
    rng0_off = ring[0].manual_sbuf_range[0]
    x_sb = at("x_sb", [64, 512], F32, offset=rng0_off)
    xT = at("xT", [128, KT, BC], F32R, offset=hsumT.manual_sbuf_range[0])
    assert off[0] <= nc.SBUF_PARTITION_SIZE_BYTES, off[0]

    # MLP working set aliases the LSTM working region (used strictly after it)
    off[0] = lstm_base
    actT = [at(f"actT{r}", [128, KT, 512], F32R) for r in range(2)]
    out1 = at("out1", [128, KT, 512], F32R)
    out2 = at("out2", [128, KT, 512], F32R)
    out3 = [at(f"out3_{m}", [128, 512], F32) for m in range(4)]
    assert off[0] <= nc.SBUF_PARTITION_SIZE_BYTES, off[0]

    with (
        nc.psum_tensor("P", [128, 4096], F32) as P,
        nc.semaphore("dma_in") as dma_in,
        nc.semaphore("dma_out") as dma_out,
        nc.semaphore("pe_s") as pe_s,
        nc.semaphore("act_s") as act_s,
        nc.semaphore("dve_s") as dve_s,
        nc.Block() as block,
    ):
        Pap = P.ap()
        G0 = Pap[0:64, 0:2048]
        G1 = Pap[0:64, 2048:4096]
        Tp = [Pap[0:128, 0:256], Pap[0:128, 2048:2304]]

        def stat_x(l, t):
            """stationary tiles ([128, BC] k-tiles) for the layer-l input."""
            if l == 1:
                return h0T
            return xT if t == 0 else hsumT

        # ---------------- SYNC: all DMA ----------------
        @block.sync
        def _(sync):
            def load(dst_ap, src_ap):
                sync.dma_start(out=dst_ap, in_=src_ap).then_inc(dma_in, 16)

            load(x_sb.ap(), x_d.ap())
            for l in range(L):
                load(wih[l].ap(), wih_d[l].ap().bitcast(F32R).rearrange("k p c -> p k c"))
                load(whh[l].ap(), whh_d[l].ap().bitcast(F32R).rearrange("k p c -> p k c"))
                load(biasr[l].ap(), bias_d[l].ap())
            load(fc1w.ap(), fc1w_d.ap().bitcast(F32R).rearrange("k p c -> p k c"))
            load(fc2w.ap(), fc2w_d.ap().bitcast(F32R).rearrange("k p c -> p k c"))
            load(fc3w.ap(), fc3w_d.ap().bitcast(F32R).rearrange("k p c -> p k c"))
            load(fc3br.ap(), fc3b_d.ap().bitcast(F32R))
            load(ones64.ap(), ones64_d.ap())
            load(ones128.ap(), ones128_d.ap().bitcast(F32R))
            load(id64.ap(), id64_d.ap())
            load(fc1b.ap(), fc1b_d.ap())
            load(fc2b.ap(), fc2b_d.ap())

            # ring flushes
            for f in range(NFLUSH):
                sync.wait_ge(dve_s, dve_v(RING * f + RING - 1, "hsum"))
                sync.dma_start(
                    out=outsT_d.ap()[:, :, RING * f:RING * (f + 1), :]
                        .rearrange("k p t b -> p k t b"),
                    in_=ring[f % 2].ap(),
                ).then_inc(dma_out, 16)

            # MLP: prefetch loads + outputs
            def mload(j):
                sync.wait_ge(dma_out, 16 * NFLUSH)
                if j >= 2:
                    sync.wait_ge(pe_s, pe_mlp(j - 2, "fc1"))
                sync.dma_start(
                    out=actT[j % 2].ap(),
                    in_=outsT_d.ap()[:, :, RING * j:RING * (j + 1), :].bitcast(F32R)
                        .rearrange("k p t b -> p k (t b)"),
                ).then_inc(dma_in, 16)

            mload(0)
            mload(1)
            for j in range(NCHUNK):
                for m in range(4):
                    sync.wait_ge(dve_s, dve_mlp(j, m))
                    tt = 8 * j + 2 * m
                    sync.dma_start(
                        out=out_d.ap()[:, tt:tt + 2, :].rearrange("b u h -> u b h"),
                        in_=out3[m].ap(),
                    ).then_inc(dma_out, 16)
                if j + 2 < NCHUNK:
                    mload(j + 2)

        # ---------------- PE ----------------
        @block.tensor
        def _(tensor):
            def bias_h_group(l, t, gp):
                """bias + recurrent-part matmuls for all 4 chunks of layer l."""
                hstat = h0T if l == 0 else h1T
                for c in range(NCH):
                    cs = slice(512 * c, 512 * (c + 1))
                    tensor.matmul(gp[:, cs], ones64.ap(), biasr[l].ap()[:, cs],
                                  start=True, stop=False)
                    if t > 0:
                        for k in range(KT):
                            tensor.matmul(gp[:, cs], hstat.ap()[:, k, :],
                                          whh[l].ap()[:, k, cs],
                                          start=False, stop=False)

            def x_group(l, t, gp):
                xstat = stat_x(l, t)
                for c in range(NCH):
                    cs = slice(512 * c, 512 * (c + 1))
                    for k in range(KT):
                        mm = tensor.matmul(gp[:, cs], xstat.ap()[:, k, :],
                                           wih[l].ap()[:, k, cs],
                                           start=False, stop=(k == KT - 1))
                    if c == 2:
                        mm.then_inc(pe_s, 1)
                mm.then_inc(pe_s, 1)

            def transpose_h(l):
                for c in range(4):
                    mm = tensor.transpose(Tp[l][:, 64 * c:64 * (c + 1)],
                                          hnew.ap()[:, 128 * c:128 * (c + 1)],
                                          id64.ap())
                mm.then_inc(pe_s, 1)

            # prologue: transpose x
            tensor.wait_ge(dma_in, DMA_IN_PRE)
            for c in range(4):
                mm = tensor.transpose(Tp[0][:, 64 * c:64 * (c + 1)],
                                      x_sb.ap()[:, 128 * c:128 * (c + 1)],
                                      id64.ap())
            mm.then_inc(pe_s, 1)

            for t in range(T):
                # L0 x-part (stationary: xT at t=0 else hsumT(t-1))
                if t == 0:
                    # bias group must come after the xT copy frees Tp[0]
                    tensor.wait_ge(dve_s, 1)
                    bias_h_group(0, 0, G0)
                else:
                    tensor.wait_ge(dve_s, dve_v(t - 1, "hsum"))
                x_group(0, t, G0)
                # L1 bias + recurrent part
                if t > 0:
                    tensor.wait_ge(act_s, act_v(t - 1, "tang1"))
                bias_h_group(1, t, G1)
                # transpose h0
                tensor.wait_ge(dve_s, dve_v(t, "h0"))
                transpose_h(0)
                # L1 x-part (stationary: h0T(t))
                tensor.wait_ge(dve_s, dve_v(t, "hT0"))
                x_group(1, t, G1)
                # transpose h1
                tensor.wait_ge(dve_s, dve_v(t, "h1"))
                transpose_h(1)
                # next step L0 bias + h-part
                if t + 1 < T:
                    tensor.wait_ge(act_s, act_v(t, "tang0"))
                    bias_h_group(0, t + 1, G0)

            # ---- MLP ----
            ps1 = [Pap[:, 512 * m:512 * (m + 1)] for m in range(4)]
            ps2 = [Pap[:, 2048 + 512 * m:2048 + 512 * (m + 1)] for m in range(4)]
            for j in range(NCHUNK):
                tensor.wait_ge(dma_in, dma_in_load(j))
                if j >= 1:
                    tensor.wait_ge(dve_s, dve_mlp(j - 1, 3))
                a = actT[j % 2]
                for m in range(4):
                    for k in range(KT):
                        mm = tensor.matmul(ps1[m], fc1w.ap()[:, k, 128 * m:128 * (m + 1)],
                                           a.ap()[:, k, :],
                                           start=(k == 0), stop=(k == KT - 1))
                mm.then_inc(pe_s, 1)
                tensor.wait_ge(act_s, act_mlp(j, 1))
                for m in range(4):
                    for k in range(KT):
                        mm = tensor.matmul(ps2[m], fc2w.ap()[:, k, 128 * m:128 * (m + 1)],
                                           out1.ap()[:, k, :],
                                           start=(k == 0), stop=(k == KT - 1))
                mm.then_inc(pe_s, 1)
                tensor.wait_ge(act_s, act_mlp(j, 2))
                for m in range(4):
                    tensor.matmul(ps1[m], ones128.ap(), fc3br.ap(),
                                  start=True, stop=False)
                    for k in range(KT):
                        mm = tensor.matmul(ps1[m],
                                           out2.ap()[:, k, 128 * m:128 * (m + 1)],
                                           fc3w.ap()[:, k, :],
                                           start=False, stop=(k == KT - 1))
                    mm.then_inc(pe_s, 1)

        # ---------------- ACT (scalar) ----------------
        @block.scalar
        def _(scalar):
            for t in range(T):
                for l in range(L):
                    gp = G0 if l == 0 else G1
                    scalar.wait_ge(pe_s, pe_v(t, f"l{l}c012"))
                    scalar.activation(sig.ap(), gp[:, 0:1536], AF.Sigmoid
                                      ).then_inc(act_s, 1)
                    scalar.wait_ge(pe_s, pe_v(t, f"l{l}c3"))
                    scalar.activation(tang.ap(), gp[:, 1536:2048], AF.Tanh
                                      ).then_inc(act_s, 1)
                    scalar.wait_ge(dve_s, dve_v(t, f"c{l}"))
                    scalar.activation(tanc.ap(), c_sb[l].ap(), AF.Tanh
                                      ).then_inc(act_s, 1)
            # MLP relu with fused per-partition bias
            for j in range(NCHUNK):
                scalar.wait_ge(pe_s, pe_mlp(j, "fc1"))
                for m in range(4):
                    a = scalar.activation(out1.ap()[:, m, :],
                                          Pap[:, 512 * m:512 * (m + 1)], AF.Relu,
                                          bias=fc1b.ap()[:, m:m + 1])
                a.then_inc(act_s, 1)
                scalar.wait_ge(pe_s, pe_mlp(j, "fc2"))
                for m in range(4):
                    a = scalar.activation(out2.ap()[:, m, :],
                                          Pap[:, 2048 + 512 * m:2048 + 512 * (m + 1)],
                                          AF.Relu, bias=fc2b.ap()[:, m:m + 1])
                a.then_inc(act_s, 1)

        # ---------------- DVE (vector) ----------------
        @block.vector
        def _(vector):
            vector.wait_ge(pe_s, 1)
            vector.tensor_copy(xT.ap().rearrange("p k b -> p (k b)"), Tp[0]
                               ).then_inc(dve_s, 1)
            for t in range(T):
                for l in range(L):
                    vector.wait_ge(act_s, act_v(t, f"tang{l}"))
                    # tmp = i * tanh(g)
                    vector.tensor_tensor(tmp.ap(), sig.ap()[:, 0:512], tang.ap(), MUL)
                    if t == 0:
                        vector.tensor_copy(c_sb[l].ap(), tmp.ap()).then_inc(dve_s, 1)
                    else:
                        # c = c*f + tmp
                        vector.tensor_tensor(c_sb[l].ap(), c_sb[l].ap(),
                                             sig.ap()[:, 512:1024], MUL)
                        vector.tensor_tensor(c_sb[l].ap(), c_sb[l].ap(), tmp.ap(),
                                             ADD).then_inc(dve_s, 1)
                    vector.wait_ge(act_s, act_v(t, f"tanc{l}"))
                    vector.tensor_tensor(hnew.ap(), sig.ap()[:, 1024:1536],
                                         tanc.ap(), MUL).then_inc(dve_s, 1)
                    vector.wait_ge(pe_s, pe_v(t, f"l{l}T"))
                    hT = h0T if l == 0 else h1T
                    vector.tensor_copy(hT.ap().rearrange("p k b -> p (k b)"), Tp[l]
                                       ).then_inc(dve_s, 1)
                # hsum + ring write
                vector.tensor_tensor(hsumT.ap(), h0T.ap(), h1T.ap(), ADD)
                blk = t // RING
                if blk >= 2:
                    vector.wait_ge(dma_out, 16 * (blk - 1))
                vector.tensor_copy(ring[blk % 2].ap()[:, :, t % RING, :],
                                   hsumT.ap()).then_inc(dve_s, 1)
            # MLP psum3 -> out3 copies
            for j in range(NCHUNK):
                for m in range(4):
                    vector.wait_ge(pe_s, pe_mlp(j, m))
                    if j >= 1:
                        vector.wait_ge(dma_out, dma_out_mlp(j - 1, m))
                    vector.tensor_copy(out3[m].ap(), Pap[:, 512 * m:512 * (m + 1)]
                                       ).then_inc(dve_s, 1)

    return nc


_PERM = None


def _gate_perm():
    # torch gate order (i, f, g, o) -> our column order (i, f, o, g)
    global _PERM
    if _PERM is None:
        i = np.arange(512)
        _PERM = np.concatenate([i, 512 + i, 1536 + i, 1024 + i])
    return _PERM


def _prep_inputs(x, W_ih, W_hh, b_ih, b_hh, fc1_w, fc1_b, fc2_w, fc2_b, fc3_w, fc3_b):
    perm = _gate_perm()
    common = {}
    for l in range(L):
        wt = np.ascontiguousarray(W_ih[l][perm].T)          # [512, 2048]
        common[f"wih{l}"] = wt.reshape(KT, 128, G)
        wt = np.ascontiguousarray(W_hh[l][perm].T)
        common[f"whh{l}"] = wt.reshape(KT, 128, G)
        common[f"bias{l}"] = (b_ih[l] + b_hh[l])[perm].reshape(1, G).astype(ml_dtypes.bfloat16)
    common["fc1w"] = np.ascontiguousarray(fc1_w.T).reshape(KT, 128, 512)
    common["fc2w"] = np.ascontiguousarray(fc2_w.T).reshape(KT, 128, 512)
    common["fc3w"] = np.ascontiguousarray(fc3_w.T).reshape(KT, 128, 512)
    common["fc1b"] = np.ascontiguousarray(fc1_b.reshape(4, 128).T)
    common["fc2b"] = np.ascontiguousarray(fc2_b.reshape(4, 128).T)
    common["fc3b"] = fc3_b.reshape(1, 512).astype(np.float32)
    common["ones64"] = np.ones((1, 64), ml_dtypes.bfloat16)
    common["ones128"] = np.ones((1, 128), np.float32)
    common["id64"] = np.eye(64, dtype=np.float32)
    in_maps = []
    for c in range(NCORES):
        m = dict(common)
        m["x"] = np.ascontiguousarray(x[BC * c:BC * (c + 1)])
        in_maps.append(m)
    return in_maps


_NC_CACHE = None


def kernel(**inputs):
    global _NC_CACHE
    if _NC_CACHE is None:
        _NC_CACHE = build_nc()
    nc = _NC_CACHE
    in_maps = _prep_inputs(**{k: np.asarray(v) for k, v in inputs.items()})
    res = run_bass_kernel_spmd(nc, in_maps, core_ids=list(range(NCORES)))
    out = np.concatenate([res.results[c]["out"] for c in range(NCORES)], axis=0)
    return out.astype(np.float32)


# revision 21
# speedup vs baseline: 2.3826x; 2.3826x over previous
"""DecoderLSTM Trainium2 kernel — 8-core data-parallel over batch.

Problem: 2-layer LSTM (H=512, B=512, T=128) where the step input is the sum of
the two layers' hidden states, followed by a 3-layer MLP head applied to the
[B, T, H] hidden-sum sequence.

Strategy (per core, B_c = 64 batch rows, zero collectives):
  - LSTM gates computed as g[B_c, 4H] with the *activations* stationary on the
    PE array ([K=128, M=64] tiles of x^T / h^T) and the *weights* streaming as
    the moving operand in fp32r (full-rate, ~1.5e-4 numerics) 512-col chunks.
  - h_new is transposed back to [H, B_c] each step with PE transpose-mode
    matmuls so the next step's stationary operands need no extra work.
  - Hidden sums are staged transposed in SBUF rings and flushed to a DRAM
    scratch every 8 steps; the MLP head then runs chunk-wise (512 rows at a
    time): fc1/fc2 weights-stationary with fused bias+ReLU on the scalar
    engine, fc3 activations-stationary so the result lands in [rows, H] layout
    for direct output DMA.
  - Raw bass (no Tile): explicit per-engine programs and semaphores.
"""

import ml_dtypes
import numpy as np

import concourse.bass as bass
import concourse.mybir as mybir
from concourse.bass_utils import run_bass_kernel_spmd

F32 = mybir.dt.float32
F32R = mybir.dt.float32r
BF16 = mybir.dt.bfloat16
AF = mybir.ActivationFunctionType
MUL = mybir.AluOpType.mult
ADD = mybir.AluOpType.add

NCORES = 8
B, H, T, L = 512, 512, 128, 2
BC = B // NCORES          # 64 batch rows per core
G = 4 * H                 # 2048 gate rows
KT = H // 128             # 4 K-tiles
NCH = 4                   # gate column chunks of 512
RING = 8                  # steps per outs ring flush
NFLUSH = T // RING        # 16
NCHUNK = (BC * T) // 512  # 16 MLP row chunks of 512

# ---- semaphore value schedules (pure functions of step/chunk) ----------------
# dve events per LSTM step: c0, h0, hT0, c1, h1, hT1, hsum  (1-based)
_DVE_E = {"c0": 1, "h0": 2, "hT0": 3, "c1": 4, "h1": 5, "hT1": 6, "hsum": 7}
# act events per LSTM step: sig0, tang0, tanc0, sig1, tang1, tanc1
_ACT_E = {"sig0": 1, "tang0": 2, "tanc0": 3, "sig1": 4, "tang1": 5, "tanc1": 6}
# pe events per LSTM step: l0c012, l0c3, l0T, l1c012, l1c3, l1T
_PE_E = {"l0c012": 1, "l0c3": 2, "l0T": 3, "l1c012": 4, "l1c3": 5, "l1T": 6}


def dve_v(t, e):
    return 1 + 7 * t + _DVE_E[e]          # +1 for initial xT copy


def act_v(t, e):
    return 6 * t + _ACT_E[e]


def pe_v(t, e):
    return 1 + 6 * t + _PE_E[e]           # +1 for x transposes


DVE_LSTM_END = 1 + 7 * T
ACT_LSTM_END = 6 * T
PE_LSTM_END = 1 + 6 * T


def dve_mlp(j, m):                        # after o3 copy (j, rowm m)
    return DVE_LSTM_END + 4 * j + m + 1


def act_mlp(j, which):                    # which: 1 = relu1, 2 = relu2
    return ACT_LSTM_END + 2 * j + which


# pe events per MLP chunk: fc1, fc2, fc3m0..fc3m3
def pe_mlp(j, e):
    base = PE_LSTM_END + 6 * j
    if e == "fc1":
        return base + 1
    if e == "fc2":
        return base + 2
    return base + 3 + int(e)


NPRE = 16                                 # preload DMA count
DMA_IN_PRE = 16 * NPRE

# per-rep semaphore totals (for benchmark builds that loop the whole program)
PE_TOT = PE_LSTM_END + 6 * NCHUNK
ACT_TOT = ACT_LSTM_END + 2 * NCHUNK
DVE_TOT = DVE_LSTM_END + 4 * NCHUNK
DMA_IN_TOT = 16 * (NPRE + NCHUNK)
DMA_OUT_TOT = 16 * (NFLUSH + 4 * NCHUNK)


def dma_in_load(j):                       # after MLP chunk-j actT load
    return DMA_IN_PRE + 16 * (j + 1)


def dma_out_flush(f):
    return 16 * (f + 1)


def dma_out_mlp(j, m):
    return 16 * (NFLUSH + 4 * j + m + 1)


def build_nc(reps=1):
    nc = bass.Bass("TRN2", target_bir_lowering=False, debug=False,
                   num_devices=NCORES)

    # ---- DRAM I/O ----
    x_d = nc.dram_tensor("x", [BC, H], F32, kind="ExternalInput")
    wih_d = [nc.dram_tensor(f"wih{l}", [KT, 128, G], F32, kind="ExternalInput") for l in range(L)]
    whh_d = [nc.dram_tensor(f"whh{l}", [KT, 128, G], F32, kind="ExternalInput") for l in range(L)]
    bias_d = [nc.dram_tensor(f"bias{l}", [1, G], BF16, kind="ExternalInput") for l in range(L)]
    fc1w_d = nc.dram_tensor("fc1w", [KT, 128, 512], F32, kind="ExternalInput")
    fc2w_d = nc.dram_tensor("fc2w", [KT, 128, 512], F32, kind="ExternalInput")
    fc3w_d = nc.dram_tensor("fc3w", [KT, 128, 512], F32, kind="ExternalInput")
    fc1b_d = nc.dram_tensor("fc1b", [128, 4], F32, kind="ExternalInput")
    fc2b_d = nc.dram_tensor("fc2b", [128, 4], F32, kind="ExternalInput")
    fc3b_d = nc.dram_tensor("fc3b", [1, 512], F32, kind="ExternalInput")
    ones64_d = nc.dram_tensor("ones64", [1, 64], BF16, kind="ExternalInput")
    ones128_d = nc.dram_tensor("ones128", [1, 128], F32, kind="ExternalInput")
    id64_d = nc.dram_tensor("id64", [64, 64], F32, kind="ExternalInput")
    outsT_d = nc.dram_tensor("outsT", [KT, 128, T, BC], F32, kind="Internal")
    out_d = nc.dram_tensor("out", [BC, T, H], F32, kind="ExternalOutput")

    # ---- hand-drawn SBUF map (per-partition byte offsets) ----
    off = [(nc.sbuf_base + 63) // 64 * 64]

    def at(name, shape, dtype, align=32, offset=None):
        o = (off[0] + align - 1) // align * align if offset is None else offset
        h = nc.alloc_sbuf_tensor_at(name, shape, dtype, offset=o)
        sz = int(np.prod(shape[1:])) * mybir.dt.size(dtype)
        if offset is None:
            off[0] = o + sz
        return h

    wih = [at(f"wih{l}s", [128, KT, G], F32R) for l in range(L)]
    whh = [at(f"whh{l}s", [128, KT, G], F32R) for l in range(L)]
    fc1w = at("fc1ws", [128, KT, 512], F32R)
    fc2w = at("fc2ws", [128, KT, 512], F32R)
    fc3w = at("fc3ws", [128, KT, 512], F32R)
    biasr = [at(f"bias{l}s", [1, G], BF16) for l in range(L)]
    fc3br = at("fc3bs", [1, 512], F32R)
    ones64 = at("ones64s", [1, 64], BF16)
    ones128 = at("ones128s", [1, 128], F32R)
    id64 = at("id64s", [64, 64], F32)
    fc1b = at("fc1bs", [128, 4], F32)
    fc2b = at("fc2bs", [128, 4], F32)

    lstm_base = off[0]
    ring = [at(f"ring{r}", [128, KT, RING, BC], F32) for r in range(2)]
    sig = at("sig", [64, 1536], F32)
    tang = at("tang", [64, 512], F32)
    tanc = at("tanc", [64, 512], F32)
    hnew = at("hnew", [64, 512], F32)
    tmp = at("tmp", [64, 512], F32)
    c_sb = [at(f"c{l}", [64, 512], F32) for l in range(L)]
    hsumT = at("hsumT", [128, KT, BC], F32R)
    h0T = at("h0T", [128, KT, BC], F32R)
    h1T = at("h1T", [128, KT, BC], F32R)
    # x_sb and xT are dead after step 0 starts: alias onto ring0 / hsumT
    rng0_off = ring[0].manual_sbuf_range[0]
    x_sb = at("x_sb", [64, 512], F32, offset=rng0_off)
    xT = at("xT", [128, KT, BC], F32R, offset=hsumT.manual_sbuf_range[0])
    assert off[0] <= nc.SBUF_PARTITION_SIZE_BYTES, off[0]

    # MLP working set aliases the LSTM working region (used strictly after it)
    off[0] = lstm_base
    actT = [at(f"actT{r}", [128, KT, 512], F32R) for r in range(2)]
    out1 = at("out1", [128, KT, 512], F32R)
    out2 = at("out2", [128, KT, 512], F32R)
    out3 = [at(f"out3_{m}", [128, 512], F32) for m in range(4)]
    assert off[0] <= nc.SBUF_PARTITION_SIZE_BYTES, off[0]

    with (
        nc.psum_tensor("P", [128, 4096], F32) as P,
        nc.semaphore("dma_in") as dma_in,
        nc.semaphore("dma_out") as dma_out,
        nc.semaphore("pe_s") as pe_s,
        nc.semaphore("act_s") as act_s,
        nc.semaphore("dve_s") as dve_s,
        nc.Block() as block,
    ):
        Pap = P.ap()
        G0 = Pap[0:64, 0:2048]
        G1 = Pap[0:64, 2048:4096]
        Tp = [Pap[0:128, 0:256], Pap[0:128, 2048:2304]]

        def stat_x(l, t):
            """stationary tiles ([128, BC] k-tiles) for the layer-l input."""
            if l == 1:
                return h0T
            return xT if t == 0 else hsumT

        # ---------------- SYNC: all DMA ----------------
        @block.sync
        def _(sync):
          for rep in range(reps):
            oD, oO, oP = rep * DVE_TOT, rep * DMA_OUT_TOT, rep * PE_TOT
            if rep > 0:
                sync.wait_ge(dma_out, rep * DMA_OUT_TOT)

            def load(dst_ap, src_ap):
                sync.dma_start(out=dst_ap, in_=src_ap).then_inc(dma_in, 16)

            load(x_sb.ap(), x_d.ap())
            for l in range(L):
                load(wih[l].ap(), wih_d[l].ap().bitcast(F32R).rearrange("k p c -> p k c"))
                load(whh[l].ap(), whh_d[l].ap().bitcast(F32R).rearrange("k p c -> p k c"))
                load(biasr[l].ap(), bias_d[l].ap())
            load(fc1w.ap(), fc1w_d.ap().bitcast(F32R).rearrange("k p c -> p k c"))
            load(fc2w.ap(), fc2w_d.ap().bitcast(F32R).rearrange("k p c -> p k c"))
            load(fc3w.ap(), fc3w_d.ap().bitcast(F32R).rearrange("k p c -> p k c"))
            load(fc3br.ap(), fc3b_d.ap().bitcast(F32R))
            load(ones64.ap(), ones64_d.ap())
            load(ones128.ap(), ones128_d.ap().bitcast(F32R))
            load(id64.ap(), id64_d.ap())
            load(fc1b.ap(), fc1b_d.ap())
            load(fc2b.ap(), fc2b_d.ap())

            # ring flushes
            for f in range(NFLUSH):
                sync.wait_ge(dve_s, oD + dve_v(RING * f + RING - 1, "hsum"))
                sync.dma_start(
                    out=outsT_d.ap()[:, :, RING * f:RING * (f + 1), :]
                        .rearrange("k p t b -> p k t b"),
                    in_=ring[f % 2].ap(),
                ).then_inc(dma_out, 16)

            # MLP: prefetch loads + outputs
            def mload(j):
                sync.wait_ge(dma_out, oO + 16 * NFLUSH)
                if j >= 2:
                    sync.wait_ge(pe_s, oP + pe_mlp(j - 2, "fc1"))
                sync.dma_start(
                    out=actT[j % 2].ap(),
                    in_=outsT_d.ap()[:, :, RING * j:RING * (j + 1), :].bitcast(F32R)
                        .rearrange("k p t b -> p k (t b)"),
                ).then_inc(dma_in, 16)

            mload(0)
            mload(1)
            for j in range(NCHUNK):
                for m in range(4):
                    sync.wait_ge(dve_s, oD + dve_mlp(j, m))
                    tt = 8 * j + 2 * m
                    sync.dma_start(
                        out=out_d.ap()[:, tt:tt + 2, :].rearrange("b u h -> u b h"),
                        in_=out3[m].ap(),
                    ).then_inc(dma_out, 16)
                if j + 2 < NCHUNK:
                    mload(j + 2)

        # ---------------- PE ----------------
        @block.tensor
        def _(tensor):
          for rep in range(reps):
            oI, oD, oA = rep * DMA_IN_TOT, rep * DVE_TOT, rep * ACT_TOT

            def bias_h_group(l, t, gp):
                """bias + recurrent-part matmuls for all 4 chunks of layer l."""
                hstat = h0T if l == 0 else h1T
                for c in range(NCH):
                    cs = slice(512 * c, 512 * (c + 1))
                    tensor.matmul(gp[:, cs], ones64.ap(), biasr[l].ap()[:, cs],
                                  start=True, stop=False)
                    if t > 0:
                        for k in range(KT):
                            tensor.matmul(gp[:, cs], hstat.ap()[:, k, :],
                                          whh[l].ap()[:, k, cs],
                                          start=False, stop=False)

            def x_group(l, t, gp):
                xstat = stat_x(l, t)
                for c in range(NCH):
                    cs = slice(512 * c, 512 * (c + 1))
                    for k in range(KT):
                        mm = tensor.matmul(gp[:, cs], xstat.ap()[:, k, :],
                                           wih[l].ap()[:, k, cs],
                                           start=False, stop=(k == KT - 1))
                    if c == 2:
                        mm.then_inc(pe_s, 1)
                mm.then_inc(pe_s, 1)

            def transpose_h(l):
                for c in range(4):
                    mm = tensor.transpose(Tp[l][:, 64 * c:64 * (c + 1)],
                                          hnew.ap()[:, 128 * c:128 * (c + 1)],
                                          id64.ap())
                mm.then_inc(pe_s, 1)

            # prologue: transpose x
            tensor.wait_ge(dma_in, oI + DMA_IN_PRE)
            for c in range(4):
                mm = tensor.transpose(Tp[0][:, 64 * c:64 * (c + 1)],
                                      x_sb.ap()[:, 128 * c:128 * (c + 1)],
                                      id64.ap())
            mm.then_inc(pe_s, 1)

            for t in range(T):
                # L0 x-part (stationary: xT at t=0 else hsumT(t-1))
                if t == 0:
                    # bias group must come after the xT copy frees Tp[0]
                    tensor.wait_ge(dve_s, oD + 1)
                    bias_h_group(0, 0, G0)
                else:
                    tensor.wait_ge(dve_s, oD + dve_v(t - 1, "hsum"))
                x_group(0, t, G0)
                # L1 bias + recurrent part
                if t > 0:
                    tensor.wait_ge(act_s, oA + act_v(t - 1, "tang1"))
                bias_h_group(1, t, G1)
                # transpose h0
                tensor.wait_ge(dve_s, oD + dve_v(t, "h0"))
                transpose_h(0)
                # L1 x-part (stationary: h0T(t))
                tensor.wait_ge(dve_s, oD + dve_v(t, "hT0"))
                x_group(1, t, G1)
                # transpose h1
                tensor.wait_ge(dve_s, oD + dve_v(t, "h1"))
                transpose_h(1)
                # next step L0 bias + h-part
                if t + 1 < T:
                    tensor.wait_ge(act_s, oA + act_v(t, "tang0"))
                    bias_h_group(0, t + 1, G0)

            # ---- MLP ----
            ps1 = [Pap[:, 512 * m:512 * (m + 1)] for m in range(4)]
            ps2 = [Pap[:, 2048 + 512 * m:2048 + 512 * (m + 1)] for m in range(4)]
            for j in range(NCHUNK):
                tensor.wait_ge(dma_in, oI + dma_in_load(j))
                if j >= 1:
                    tensor.wait_ge(dve_s, oD + dve_mlp(j - 1, 3))
                a = actT[j % 2]
                for m in range(4):
                    for k in range(KT):
                        mm = tensor.matmul(ps1[m], fc1w.ap()[:, k, 128 * m:128 * (m + 1)],
                                           a.ap()[:, k, :],
                                           start=(k == 0), stop=(k == KT - 1))
                mm.then_inc(pe_s, 1)
                tensor.wait_ge(act_s, oA + act_mlp(j, 1))
                for m in range(4):
                    for k in range(KT):
                        mm = tensor.matmul(ps2[m], fc2w.ap()[:, k, 128 * m:128 * (m + 1)],
                                           out1.ap()[:, k, :],
                                           start=(k == 0), stop=(k == KT - 1))
                mm.then_inc(pe_s, 1)
                tensor.wait_ge(act_s, oA + act_mlp(j, 2))
                for m in range(4):
                    tensor.matmul(ps1[m], ones128.ap(), fc3br.ap(),
                                  start=True, stop=False)
                    for k in range(KT):
                        mm = tensor.matmul(ps1[m],
                                           out2.ap()[:, k, 128 * m:128 * (m + 1)],
                                           fc3w.ap()[:, k, :],
                                           start=False, stop=(k == KT - 1))
                    mm.then_inc(pe_s, 1)

        # ---------------- ACT (scalar) ----------------
        @block.scalar
        def _(scalar):
          for rep in range(reps):
            oP, oD = rep * PE_TOT, rep * DVE_TOT
            for t in range(T):
                for l in range(L):
                    gp = G0 if l == 0 else G1
                    scalar.wait_ge(pe_s, oP + pe_v(t, f"l{l}c012"))
                    scalar.activation(sig.ap(), gp[:, 0:1536], AF.Sigmoid
                                      ).then_inc(act_s, 1)
                    scalar.wait_ge(pe_s, oP + pe_v(t, f"l{l}c3"))
                    scalar.activation(tang.ap(), gp[:, 1536:2048], AF.Tanh
                                      ).then_inc(act_s, 1)
                    scalar.wait_ge(dve_s, oD + dve_v(t, f"c{l}"))
                    scalar.activation(tanc.ap(), c_sb[l].ap(), AF.Tanh
                                      ).then_inc(act_s, 1)
            # MLP relu with fused per-partition bias
            for j in range(NCHUNK):
                scalar.wait_ge(pe_s, oP + pe_mlp(j, "fc1"))
                for m in range(4):
                    a = scalar.activation(out1.ap()[:, m, :],
                                          Pap[:, 512 * m:512 * (m + 1)], AF.Relu,
                                          bias=fc1b.ap()[:, m:m + 1])
                a.then_inc(act_s, 1)
                scalar.wait_ge(pe_s, oP + pe_mlp(j, "fc2"))
                for m in range(4):
                    a = scalar.activation(out2.ap()[:, m, :],
                                          Pap[:, 2048 + 512 * m:2048 + 512 * (m + 1)],
                                          AF.Relu, bias=fc2b.ap()[:, m:m + 1])
                a.then_inc(act_s, 1)

        # ---------------- DVE (vector) ----------------
        @block.vector
        def _(vector):
          for rep in range(reps):
            oP, oA, oO = rep * PE_TOT, rep * ACT_TOT, rep * DMA_OUT_TOT
            vector.wait_ge(pe_s, oP + 1)
            vector.tensor_copy(xT.ap().rearrange("p k b -> p (k b)"), Tp[0]
                               ).then_inc(dve_s, 1)
            for t in range(T):
                for l in range(L):
                    vector.wait_ge(act_s, oA + act_v(t, f"tang{l}"))
                    # tmp = i * tanh(g)
                    vector.tensor_tensor(tmp.ap(), sig.ap()[:, 0:512], tang.ap(), MUL)
                    if t == 0:
                        vector.tensor_copy(c_sb[l].ap(), tmp.ap()).then_inc(dve_s, 1)
                    else:
                        # c = c*f + tmp
                        vector.tensor_tensor(c_sb[l].ap(), c_sb[l].ap(),
                                             sig.ap()[:, 512:1024], MUL)
                        vector.tensor_tensor(c_sb[l].ap(), c_sb[l].ap(), tmp.ap(),
                                             ADD).then_inc(dve_s, 1)
                    vector.wait_ge(act_s, oA + act_v(t, f"tanc{l}"))
                    vector.tensor_tensor(hnew.ap(), sig.ap()[:, 1024:1536],
                                         tanc.ap(), MUL).then_inc(dve_s, 1)
                    vector.wait_ge(pe_s, oP + pe_v(t, f"l{l}T"))
                    hT = h0T if l == 0 else h1T
                    vector.tensor_copy(hT.ap().rearrange("p k b -> p (k b)"), Tp[l]
                                       ).then_inc(dve_s, 1)
                # hsum + ring write
                vector.tensor_tensor(hsumT.ap(), h0T.ap(), h1T.ap(), ADD)
                blk = t // RING
                if blk >= 2:
                    vector.wait_ge(dma_out, oO + 16 * (blk - 1))
                vector.tensor_copy(ring[blk % 2].ap()[:, :, t % RING, :],
                                   hsumT.ap()).then_inc(dve_s, 1)
            # MLP psum3 -> out3 copies
            for j in range(NCHUNK):
                for m in range(4):
                    vector.wait_ge(pe_s, oP + pe_mlp(j, m))
                    if j >= 1:
                        vector.wait_ge(dma_out, oO + dma_out_mlp(j - 1, m))
                    vector.tensor_copy(out3[m].ap(), Pap[:, 512 * m:512 * (m + 1)]
                                       ).then_inc(dve_s, 1)

    return nc


_PERM = None


def _gate_perm():
    # torch gate order (i, f, g, o) -> our column order (i, f, o, g)
    global _PERM
    if _PERM is None:
        i = np.arange(512)
        _PERM = np.concatenate([i, 512 + i, 1536 + i, 1024 + i])
    return _PERM


def _prep_inputs(x, W_ih, W_hh, b_ih, b_hh, fc1_w, fc1_b, fc2_w, fc2_b, fc3_w, fc3_b):
    perm = _gate_perm()
    common = {}
    for l in range(L):
        wt = np.ascontiguousarray(W_ih[l][perm].T)          # [512, 2048]
        common[f"wih{l}"] = wt.reshape(KT, 128, G)
        wt = np.ascontiguousarray(W_hh[l][perm].T)
        common[f"whh{l}"] = wt.reshape(KT, 128, G)
        common[f"bias{l}"] = (b_ih[l] + b_hh[l])[perm].reshape(1, G).astype(ml_dtypes.bfloat16)
    common["fc1w"] = np.ascontiguousarray(fc1_w.T).reshape(KT, 128, 512)
    common["fc2w"] = np.ascontiguousarray(fc2_w.T).reshape(KT, 128, 512)
    common["fc3w"] = np.ascontiguousarray(fc3_w.T).reshape(KT, 128, 512)
    common["fc1b"] = np.ascontiguousarray(fc1_b.reshape(4, 128).T)
    common["fc2b"] = np.ascontiguousarray(fc2_b.reshape(4, 128).T)
    common["fc3b"] = fc3_b.reshape(1, 512).astype(np.float32)
    common["ones64"] = np.ones((1, 64), ml_dtypes.bfloat16)
    common["ones128"] = np.ones((1, 128), np.float32)
    common["id64"] = np.eye(64, dtype=np.float32)
    in_maps = []
    for c in range(NCORES):
        m = dict(common)
        m["x"] = np.ascontiguousarray(x[BC * c:BC * (c + 1)])
        in_maps.append(m)
    return in_maps


_NC_CACHE = None


def kernel(**inputs):
    global _NC_CACHE
    if _NC_CACHE is None:
        _NC_CACHE = build_nc()
    nc = _NC_CACHE
    in_maps = _prep_inputs(**{k: np.asarray(v) for k, v in inputs.items()})
    res = run_bass_kernel_spmd(nc, in_maps, core_ids=list(range(NCORES)))
    out = np.concatenate([res.results[c]["out"] for c in range(NCORES)], axis=0)
    return out.astype(np.float32)
